# revision 49
# baseline (speedup 1.0000x reference)
"""Depth-modulated 3x3 conv (DepthConv2d) Trainium2 Bass kernel, v3.

Math (per batch image, P = zero-padded image, k = 3i+j):
  out[o, y, x] = bias[o] + sum_{c,k} w[o,c,k] * P[c, y+i, x+j] * sim[k, y, x]
  sim[k, y, x] = exp(-8.3 * |Pd[y+i, x+j] - Pd[y, x]|)   (sim[0] == 1)

Strategy (8 cores, one image per core):
  - Offsets paired for K=128 matmuls with 4B-aligned, fully bf16 DVE
    modulates:
      Ta = [img<<1 down-right pad ; img pad] stacks, serving pairs
      (1,2), (4,5), (7,8) at row offsets g=0,1,2 -- ONE fused DVE
      tensor_tensor (2x_1p mode) modulates all three groups;
      Tb serves pair (3,6) plus the unmodulated k=0 term (K=64 matmul).
  - sim rows are packed as bf16 PIXEL PAIRS inside fp32 words; the
    replication matmul (fp32, K=2, 1.0/0.0 selection lhsT, row-tiled
    over 4 PE row groups) bit-copies the words into PSUM, which the DVE
    then reads back as bf16 (2x_1p fast mode) for the modulates.
  - Host pre-pads the image into three shifted plane-contiguous copies
    so every image DMA moves ~9KB contiguous descriptor runs; output is
    dumped partition-major and reassembled on host.
  - Main matmuls bf16; even/odd tiles share one PSUM bank via
    col-tiling (tile_position=(0,0)/(0,64)); ACT adds bias per odd tile.
"""

import numpy as np
import ml_dtypes

import concourse.bass as bass
import concourse.mybir as mybir
import concourse.tile as tile
from concourse.bass_utils import run_bass_kernel_spmd
import bass_rust

F32 = mybir.dt.float32
BF16 = mybir.dt.bfloat16
BF = ml_dtypes.bfloat16
ALPHA = 8.3
N_CORES = 8

_WAIT_CAP = 1  # walrus engine-instruction sync-wait slot limit
_EV_CAP = 2  # InstEventSemaphore holds up to 2 waits


def _split_excess_waits(nc):
    """Move excess sync waits (>_WAIT_CAP) off engine instructions onto
    standalone InstEventSemaphore carriers inserted just before, on the same
    engine.  Tile's scheduler often leaves 2+ waits on one instruction,
    which walrus codegen rejects ("Too many sync wait commands")."""
    for bb in nc.main_func.blocks:
        out = []
        for ins in bb.instructions:
            si = ins.sync_info
            tname = type(ins).__name__
            if (
                si is not None
                and si.on_wait
                and len(si.on_wait) > _WAIT_CAP
                and tname != "InstEventSemaphore"
            ):
                waits = list(si.on_wait)
                keep = waits[-_WAIT_CAP:]
                excess = waits[:-_WAIT_CAP]
                while excess:
                    chunk, excess = excess[:_EV_CAP], excess[_EV_CAP:]
                    ev = bass_rust.InstEventSemaphore(
                        name=nc.get_next_instruction_name(), ins=[], outs=[]
                    )
                    ev.engine = ins.engine
                    ev.sync_info = bass_rust.SyncInfo(on_wait=chunk, on_update=[])
                    out.append(ev)
                si.on_wait = keep
            out.append(ins)
        bb.instructions[:] = out


def _ov(apobj, pattern):
    """Copy an AP and overwrite its access pattern ([stride, num] pairs,
    partition dim first).  Used for overlapping read windows the slicing
    API cannot express."""
    c = apobj.copy()
    c.ap = bass_rust.VecI64Pair([list(p) for p in pattern])
    return c


def build_bass(
    split_waits: bool = True, rings: str = "css", ntiles: int = 32, skip: tuple = ()
):
    """rings: 3 chars for (small-loads ring, Tb/spw alt ring, spw alt ring):
    's'=sync, 'c'=scalar, 'g'=gpsimd."""
    nc = bass.Bass()

    def _eng(ch):
        return {"s": nc.sync, "c": nc.scalar, "g": nc.gpsimd}[ch]
    # imgpad planes (bf16, plane-contiguous):
    #   plane 0 (L): L[r, x] = P[r-1, x-1]   (img at [2:130, 2:130])
    #   plane 1 (U): U[r, x] = P[r-1, x]     (img at [2:130, 1:129])
    #   plane 2 (D): D[r, x] = P[r,   x]     (img at [1:129, 1:129])
    img_d = nc.dram_tensor("imgpad", [3, 64, 133, 132], BF16, kind="ExternalInput")
    # dpad3[y, i, x] = Pd[y+i, x]  (padded depth rows, f32)
    dep_d = nc.dram_tensor("dpad3", [128, 3, 130], F32, kind="ExternalInput")
    # packed small operands: wb = [wp(4*64) | w0(64)] bf16; wf = [sel(128) | bias(1)]
    wb_d = nc.dram_tensor("wb", [128, 320], BF16, kind="ExternalInput")
    wf_d = nc.dram_tensor("wf", [128, 129], F32, kind="ExternalInput")
    out_d = nc.dram_tensor("out", [128, 2, 4096], BF16, kind="ExternalOutput")

    with tile.TileContext(nc) as tc:
        with (
            tc.tile_pool(name="singles", bufs=1) as singles,
            tc.tile_pool(name="dram", bufs=1, space="DRAM") as drampool,
            tc.tile_pool(name="repp", bufs=1, space="PSUM") as repp,
            tc.tile_pool(name="outps", bufs=3, space="PSUM") as outps,
            tc.tile_pool(name="mods", bufs=4) as mods,
            tc.tile_pool(name="outsb", bufs=2) as outsb,
        ):
            e1, e2, e3 = _eng(rings[0]), _eng(rings[1]), _eng(rings[2])
            # ---- depth FIRST (on the image ring, ahead of the bulk), then
            # packed small operands on ring e1
            Dt = singles.tile([128, 3, 130], F32)
            nc.sync.dma_start(out=Dt[:], in_=dep_d[:])
            wb_sb = singles.tile([128, 320], BF16)
            wf_sb = singles.tile([128, 129], F32)
            e1.dma_start(out=wb_sb[:], in_=wb_d[:])
            e1.dma_start(out=wf_sb[:], in_=wf_d[:])

            # ---- image stacks.
            # Ta[p<64]  = L rows 32b+1.. : P[32b+r, x-1] (j=1 view at x0=2)
            # Ta[p>=64] = U rows 32b+1.. : P[32b+r, x]   (j=2 view at x0=2)
            # Tb[p<64]  = U rows 32b+1.. : P[32b+r, x]   (k3/k0 at x0=0)
            # Tb[p>=64] = D rows 32b+1.. : P[32b+r+1, x] (k6 at x0=0)
            Ta = singles.tile([128, 4, 34, 132], BF16, name="Ta")
            Tb = singles.tile([128, 4, 34, 132], BF16, name="Tb")
            for b in range(4):
                nc.sync.dma_start(
                    out=Ta[:, b],
                    in_=img_d[0:2, :, 32 * b + 1 : 32 * b + 35, :].rearrange(
                        "h c r x -> (h c) r x"
                    ),
                )
                eng = nc.sync if b % 2 else e2
                eng.dma_start(
                    out=Tb[:, b],
                    in_=img_d[1:3, :, 32 * b + 1 : 32 * b + 35, :].rearrange(
                        "h c r x -> (h c) r x"
                    ),
                )

            # ---- sim (k=1..8) in [y, k, x]: exp(-a*|Pd[y+i,x+j] - Pd[y,x]|)
            simf = singles.tile([128, 8, 128], F32)
            for k in range(1, 9):
                i, j = k // 3, k % 3
                nc.vector.tensor_tensor(
                    out=simf[:, k - 1, :],
                    in0=Dt[:, i, j : j + 128],
                    in1=Dt[:, 0, 0:128],
                    op=mybir.AluOpType.subtract,
                )
            nc.scalar.activation(
                out=simf[:, :, :],
                in_=simf[:, :, :],
                func=mybir.ActivationFunctionType.Abs,
                scale=ALPHA,
            )
            sim_bf = singles.tile([128, 8, 128], BF16)
            nc.scalar.activation(
                out=sim_bf[:, :, :],
                in_=simf[:, :, :],
                func=mybir.ActivationFunctionType.Exp,
                scale=-1.0,
            )
            # spw rows (bf16 pixel pairs packed in fp32 words):
            #   32g+0 = sim for lower half, 32g+1 = upper half
            #   g<3: k = 3g+1+m ; g=3: k = 3, 6
            spw = singles.tile([128, 8192], F32)
            spw_bf = spw[:].bitcast(BF16)  # [128, 16384]
            sim_dram = drampool.tile([8, 16384], BF16)
            e1.dma_start(
                out=sim_dram[:].rearrange("k (y x) -> y k x", y=128),
                in_=sim_bf[:],
            )
            for g in range(4):
                if g < 3:
                    src = sim_dram[3 * g : 3 * g + 2, :]  # k = 3g+1, 3g+2
                else:
                    src = sim_dram[2:6:3, :]  # k = 3, 6
                eng = [e1, e3, e1, e3][g]
                eng.dma_start(out=spw_bf[32 * g : 32 * g + 2, :], in_=src)

            # ---- main loop: 16 pairs of tiles (4 image rows, 512 px
            # each tile).  Even/odd main matmuls alternate so their
            # disjoint PE column groups (h0/h64) overlap in the array.
            for T in range(ntiles // 2):
                t0 = 2 * T
                b = t0 // 8
                yl0 = 4 * t0 - 32 * b
                yl1 = yl0 + 4

                # one full PSUM bank per rep matmul (concurrent row-tiled
                # matmuls must not share a bank); each rep covers the pair
                rep = repp.tile([128, 4, 512], F32, tag="rep", name=f"rep{T}")
                for g in range(4):
                    if "rep" in skip:
                        continue
                    nc.tensor.matmul(
                        out=rep[:, g, :],
                        lhsT=wf_sb[32 * g : 32 * g + 2, 0:128],
                        rhs=spw[32 * g : 32 * g + 2, 512 * T : 512 * T + 512],
                        start=True,
                        stop=True,
                        tile_position=(32 * g, 0),
                    )
                rep_bf = rep[:].bitcast(BF16)  # [128, 4, 1024]

                out_ps = outps.tile([128, 4, 128], F32, tag="outps", name=f"o{T}")
                # k=0 terms first (depend only on Tb): fill the PE gap
                # while the modulates for this pair are still running
                for h, yl in ((0, yl0), (1, yl1)):
                    nc.tensor.matmul(
                        out=out_ps[64 * h : 64 * h + 64],
                        lhsT=wb_sb[0:64, 256:320],
                        rhs=Tb[0:64, b, yl : yl + 4, 0:128],
                        start=True,
                        stop=("main" in skip),
                        tile_position=(0, 64 * h),
                    )

                es = 4 * 34 * 132  # elements per partition in Ta
                base_pat = [[es, 128], [132, 3], [132, 4], [1, 128]]
                ms = []
                for h, yl in ((0, yl0), (1, yl1)):
                    m = mods.tile(
                        [128, 4, 4, 128], BF16, tag=f"mod{h}", name=f"m{T}_{h}"
                    )
                    ms.append(m)
                    if "mod" in skip:
                        nc.vector.memset(m[:], 0.25)
                        continue
                    nc.vector.tensor_tensor(
                        out=m[:, 0:3],
                        in0=_ov(Ta[:, b, yl : yl + 4, 2:130], base_pat),
                        in1=rep_bf[:, 0:3, 512 * h : 512 * h + 512].rearrange(
                            "p g (r x) -> p g r x", r=4
                        ),
                        op=mybir.AluOpType.mult,
                    )
                for h, yl in ((0, yl0), (1, yl1)):
                    if "mod" in skip:
                        continue
                    nc.vector.tensor_tensor(
                        out=ms[h][:, 3],
                        in0=Tb[:, b, yl + 1 : yl + 5, 0:128],
                        in1=rep_bf[:, 3, 512 * h : 512 * h + 512].rearrange(
                            "p (r x) -> p r x", r=4
                        ),
                        op=mybir.AluOpType.mult,
                    )

                for g in range(4):
                    if "main" in skip:
                        continue
                    for h in (0, 1):
                        nc.tensor.matmul(
                            out=out_ps[64 * h : 64 * h + 64],
                            lhsT=wb_sb[:, 64 * g : 64 * g + 64],
                            rhs=ms[h][:, g],
                            start=False,
                            stop=(g == 3),
                            tile_position=(0, 64 * h),
                        )

                if T % 8 == 0:
                    out_sb = outsb.tile(
                        [128, 8, 4, 128], BF16, tag="outsb", name=f"os{T}"
                    )
                q = T % 8
                nc.scalar.activation(
                    out=out_sb[:, q, :, :],
                    in_=out_ps[:],
                    func=mybir.ActivationFunctionType.Identity,
                    bias=wf_sb[:, 128:129],
                    scale=1.0,
                )
                if T % 8 == 7:
                    nc.sync.dma_start(
                        out=out_d[:, T // 8, :],
                        in_=out_sb[:].rearrange("p q r x -> p (q r x)"),
                    )
    if split_waits:
        _split_excess_waits(nc)
    return nc


_NC_CACHE = None


def _get_nc():
    global _NC_CACHE
    if _NC_CACHE is None:
        _NC_CACHE = build_bass()
    return _NC_CACHE


def _prep_operands(weight, bias):
    wtk = weight.reshape(64, 64, 9)  # w[o, c, k]
    wb = np.zeros((128, 320), BF)
    for g in range(3):
        wb[0:64, 64 * g : 64 * g + 64] = wtk[:, :, 3 * g + 1].T.astype(BF)  # j=1
        wb[64:128, 64 * g : 64 * g + 64] = wtk[:, :, 3 * g + 2].T.astype(BF)  # j=2
    wb[0:64, 192:256] = wtk[:, :, 3].T.astype(BF)  # lower = k3
    wb[64:128, 192:256] = wtk[:, :, 6].T.astype(BF)  # upper = k6
    wb[0:64, 256:320] = wtk[:, :, 0].T.astype(BF)  # k0
    wf = np.zeros((128, 129), np.float32)
    for g in range(4):
        wf[32 * g, 0:64] = 1.0
        wf[32 * g + 1, 64:128] = 1.0
    wf[:, 128] = np.concatenate([bias, bias]).astype(np.float32)
    return wb, wf


def _prep_image(img_bf):
    """img_bf: [64, 128, 128] bf16 -> padded shifted planes [3, 64, 133, 132]."""
    Z = np.zeros((3, 64, 133, 132), BF)
    Z[0, :, 2:130, 2:130] = img_bf  # L: P[r-1, x-1]
    Z[1, :, 2:130, 1:129] = img_bf  # U: P[r-1, x]
    Z[2, :, 1:129, 1:129] = img_bf  # D: P[r, x]
    return Z


def _prep_depth(dep):
    """dep: [128, 128] f32 -> dpad3[y, i, x] = Pd[y+i, x], [128, 3, 130]."""
    Pd = np.zeros((131, 130), np.float32)
    Pd[1:129, 1:129] = dep
    out = np.empty((128, 3, 130), np.float32)
    for i in range(3):
        out[:, i, :] = Pd[i : i + 128, :]
    return out


def kernel(image, depth, weight, bias, **kwargs):
    image = np.asarray(image, dtype=np.float32)
    depth = np.ascontiguousarray(np.asarray(depth, dtype=np.float32))
    weight = np.ascontiguousarray(np.asarray(weight, dtype=np.float32))
    bias = np.ascontiguousarray(np.asarray(bias, dtype=np.float32))

    B = image.shape[0]
    assert B == N_CORES, f"expected batch {N_CORES}, got {B}"

    wb, wf = _prep_operands(weight, bias)
    image_bf = image.astype(BF)

    global _last_in_maps
    nc = _get_nc()
    in_maps = [
        {
            "imgpad": _prep_image(image_bf[b]),
            "dpad3": _prep_depth(depth[b, 0]),
            "wb": wb,
            "wf": wf,
        }
        for b in range(B)
    ]
    _last_in_maps = in_maps
    res = run_bass_kernel_spmd(nc, in_maps, core_ids=list(range(N_CORES)))
    # out[p, B2, q, r, x]: p = h*64 + c (h=0 even tile, h=1 odd);
    # image row = 64*B2 + 8*q + 4*h + r
    outs = []
    for r in res.results:
        v = r["out"].reshape(2, 64, 2, 8, 4, 128)  # h c B2 q r x
        full = np.transpose(v, (1, 2, 3, 0, 4, 5)).reshape(64, 128, 128)
        outs.append(full)
    return np.stack(outs, axis=0).astype(np.float32)
